# revision 1
# baseline (speedup 1.0000x reference)
"""Trainium2 Bass kernel for nn_ComplexCrossAttention.

Strategy:
- Data-parallel over batch B=8 across 8 NeuronCores (one batch element each,
  no collectives).
- Complex linears are folded into single real matmuls on stacked
  real/imag feature-major activations Z = [re; im] with host-prestacked
  weights [[Wr^T, Wi^T], [-Wi^T, Wr^T]]  (out = W_stack^T-contract over 2*Din).
- Attention per head: scores are computed TRANSPOSED (St[k,q]) so softmax-exp
  runs straight out of PSUM with no transposes; the key-axis softmax sum is a
  ones-vector matmul on the PE; normalization by 1/denom is deferred to the
  AV-output eviction (per-column broadcast multiply).
- exp() needs no max-subtraction for this problem's input distribution
  (|scores| < ~10 << 88).
- All matmuls run as float32r (full-rate fp32 on the PE; moving dim kept
  >= 256).
"""

import sys

for _p in ("/opt/trn_rl_repo",):
    if _p not in sys.path:
        sys.path.insert(0, _p)

import numpy as np

import concourse.bass as bass
import concourse.mybir as mybir
import concourse.tile as tile
from concourse import bacc
from concourse.bass_utils import run_bass_kernel_spmd

FP32R = mybir.dt.float32r
FP32 = mybir.dt.float32
AF = mybir.ActivationFunctionType
OP = mybir.AluOpType

B, S, D = 8, 512, 1024
NH, DH = 16, 64
HID = 4096
T = S
N_CORES = 8
D2 = 2 * D       # 2048 stacked features
H2 = 2 * HID     # 8192 stacked hidden
KC_D = D2 // 128   # 16 contraction chunks of the model dim
MC_D = D2 // 128   # 16 output chunks of the model dim
MC_H = H2 // 128   # 64 chunks of the hidden dim
EPS = 1e-5

# fc/proj hidden chunk order: [re half1, im half1, re half2, im half2] so each
# token-independent MLP "hidden half" is a contiguous chunk range pairing
# re chunk j with im chunk j+16.
MC_ORDER = (
    list(range(0, 16)) + list(range(32, 48))
    + list(range(16, 32)) + list(range(48, 64))
)


def _build_nc():
    nc = bacc.Bacc(None, target_bir_lowering=False, debug=False)

    zq_d = nc.dram_tensor("zq", [KC_D, 128, T], FP32R, kind="ExternalInput")
    zx_d = nc.dram_tensor("zx", [KC_D, 128, T], FP32R, kind="ExternalInput")
    wq_d = nc.dram_tensor("wq", [MC_D, 128, KC_D, 128], FP32R, kind="ExternalInput")
    wk_d = nc.dram_tensor("wk", [MC_D, 128, KC_D, 128], FP32R, kind="ExternalInput")
    wv_d = nc.dram_tensor("wv", [KC_D, 128, D2], FP32R, kind="ExternalInput")
    wfc_d = nc.dram_tensor("wfc", [MC_H, 128, KC_D, 128], FP32R, kind="ExternalInput")
    wpj_d = nc.dram_tensor("wpj", [MC_D, 128, MC_H, 128], FP32R, kind="ExternalInput")
    bq_d = nc.dram_tensor("bq", [MC_D, 128], FP32, kind="ExternalInput")
    bk_d = nc.dram_tensor("bk", [MC_D, 128], FP32, kind="ExternalInput")
    bv_d = nc.dram_tensor("bv", [1, D2], FP32, kind="ExternalInput")
    bfc_d = nc.dram_tensor("bfc", [MC_H, 128], FP32, kind="ExternalInput")
    bp_d = nc.dram_tensor("bp", [MC_D, 128], FP32, kind="ExternalInput")
    lng_d = nc.dram_tensor("lng", [128, 48], FP32, kind="ExternalInput")
    lnb_d = nc.dram_tensor("lnb", [128, 48], FP32, kind="ExternalInput")
    y_d = nc.dram_tensor("y", [MC_D, 128, T], FP32R, kind="ExternalOutput")

    with tile.TileContext(nc) as tc:
        consts_cm = tc.tile_pool(name="consts", bufs=1)
        consts = consts_cm.__enter__()

        ones_f = consts.tile([128, 1], FP32)
        nc.vector.memset(ones_f[:], 1.0)
        eps_t = consts.tile([128, 1], FP32)
        nc.vector.memset(eps_t[:], EPS)
        ones_r = consts.tile([128, 1], FP32R)
        nc.vector.tensor_copy(ones_r[:], ones_f[:])
        bq_s = consts.tile([128, MC_D], FP32)
        nc.sync.dma_start(bq_s[:], bq_d.rearrange("m p -> p m"))
        bk_s = consts.tile([128, MC_D], FP32)
        nc.sync.dma_start(bk_s[:], bk_d.rearrange("m p -> p m"))
        bfc_s = consts.tile([128, MC_H], FP32)
        nc.sync.dma_start(bfc_s[:], bfc_d.rearrange("m p -> p m"))
        bp_s = consts.tile([128, MC_D], FP32)
        nc.sync.dma_start(bp_s[:], bp_d.rearrange("m p -> p m"))
        bv_row = consts.tile([1, D2], FP32)
        nc.sync.dma_start(bv_row[:], bv_d[:])
        bv_b = consts.tile([128, D2], FP32)
        nc.gpsimd.partition_broadcast(bv_b[:], bv_row[:])
        lng_s = consts.tile([128, 48], FP32)
        nc.sync.dma_start(lng_s[:], lng_d[:])
        lnb_s = consts.tile([128, 48], FP32)
        nc.sync.dma_start(lnb_s[:], lnb_d[:])

        def ln_gb(idx, comp, c8):
            j = idx * 16 + comp * 8 + c8
            return lng_s[:, j:j + 1], lnb_s[:, j:j + 1]

        # ---- long-lived activation pools (manually scoped) ----
        zx_cm = tc.tile_pool(name="zx", bufs=1)
        zx_pool = zx_cm.__enter__()
        zx_s = zx_pool.tile([128, KC_D, T], FP32R, name="zx_s")
        nc.sync.dma_start(zx_s[:], zx_d.rearrange("c p t -> p c t"))

        o_cm = tc.tile_pool(name="op", bufs=1)
        o_pool = o_cm.__enter__()
        o_s = o_pool.tile([128, MC_D, T], FP32R, name="o_s")

        q_cm = tc.tile_pool(name="qp", bufs=1)
        q_pool = q_cm.__enter__()
        q_s = q_pool.tile([128, NH, T], FP32R, name="q_s")

        # =============== Phase A: Q projection (feature-major) ===============
        with (
            tc.tile_pool(name="zqa", bufs=1) as zqa_pool,
            tc.tile_pool(name="wqp", bufs=3) as wq_pool,
            tc.tile_pool(name="psA", bufs=4, space="PSUM") as psA,
        ):
            zq_a = zqa_pool.tile([128, KC_D, T], FP32R, name="zq_a")
            nc.sync.dma_start(zq_a[:], zq_d.rearrange("c p t -> p c t"))
            for mc in range(MC_D):
                wt = wq_pool.tile([128, KC_D, 128], FP32R, tag="wq")
                nc.sync.dma_start(wt[:], wq_d[mc])
                ps = psA.tile([128, T], FP32, tag="psA")
                for kc in range(KC_D):
                    nc.tensor.matmul(
                        ps[:], wt[:, kc, :], zq_a[:, kc, :],
                        start=(kc == 0), stop=(kc == KC_D - 1),
                    )
                nc.scalar.activation(
                    q_s[:, mc, :], ps[:], AF.Identity, bias=bq_s[:, mc:mc + 1]
                )

        # =============== Phase B: attention, head-streamed ===============
        with (
            tc.tile_pool(name="wkp", bufs=2) as wk_pool,
            tc.tile_pool(name="wvp", bufs=1) as wv_pool,
            tc.tile_pool(name="kp", bufs=4) as k_pool,
            tc.tile_pool(name="vp", bufs=2) as v_pool,
            tc.tile_pool(name="ep", bufs=10) as e_pool,
            tc.tile_pool(name="stp", bufs=2) as st_pool,
            tc.tile_pool(name="ttp", bufs=1) as tt_pool,
            tc.tile_pool(name="recp", bufs=2) as rec_pool,
            tc.tile_pool(name="bcp", bufs=2) as bc_pool,
            tc.tile_pool(name="psK", bufs=1, space="PSUM") as psK,
            tc.tile_pool(name="psV", bufs=1, space="PSUM") as psV,
            tc.tile_pool(name="psS", bufs=2, space="PSUM") as psS,
            tc.tile_pool(name="psO", bufs=2, space="PSUM") as psO,
            tc.tile_pool(name="psD", bufs=2, space="PSUM") as psD,
        ):
            v_cur = None
            for h in range(NH):
                hp, par = divmod(h, 2)
                if par == 0:
                    # V1 projection for the head pair (token-major) + V2 assembly
                    wvt = wv_pool.tile([128, KC_D, 256], FP32R, tag="wv")
                    nc.sync.dma_start(
                        wvt[:],
                        wv_d[:, :, hp * 256:(hp + 1) * 256].rearrange("c p f -> p c f"),
                    )
                    v_cur = v_pool.tile([128, 4, 512], FP32R, tag="v")
                    for tcb in range(4):
                        psv = psV.tile([128, 256], FP32, tag="psV")
                        for kc in range(KC_D):
                            nc.tensor.matmul(
                                psv[:],
                                zx_s[:, kc, tcb * 128:(tcb + 1) * 128],
                                wvt[:, kc, :],
                                start=(kc == 0), stop=(kc == KC_D - 1),
                            )
                        for sub in range(2):
                            hh = hp * 2 + sub
                            base = sub * 256
                            nc.vector.tensor_tensor(
                                v_cur[:, tcb, base:base + 128],
                                psv[:, sub * 128:(sub + 1) * 128],
                                bv_b[:, hh * 128:(hh + 1) * 128],
                                OP.add,
                            )
                            # V2 = [-Vi | Vr]
                            nc.vector.tensor_scalar_mul(
                                v_cur[:, tcb, base + 128:base + 192],
                                v_cur[:, tcb, base + 64:base + 128],
                                -1.0,
                            )
                            nc.vector.tensor_copy(
                                v_cur[:, tcb, base + 192:base + 256],
                                v_cur[:, tcb, base:base + 64],
                            )

                # K1 = [Kr; -Ki] projection (feature-major); K2 = [Ki; Kr]
                # is a partition swap + negate of K1 (saves 16 matmuls/head)
                wkt = wk_pool.tile([128, KC_D, 128], FP32R, tag="wk")
                nc.sync.dma_start(wkt[:], wk_d[h])
                k1 = k_pool.tile([128, T], FP32R, tag="k")
                ps = psK.tile([128, T], FP32, tag="psK")
                for kc in range(KC_D):
                    nc.tensor.matmul(
                        ps[:], wkt[:, kc, :], zx_s[:, kc, :],
                        start=(kc == 0), stop=(kc == KC_D - 1),
                    )
                nc.scalar.activation(
                    k1[:], ps[:], AF.Identity, bias=bk_s[:, h:h + 1]
                )
                k2 = k_pool.tile([128, T], FP32R, tag="k")
                nc.sync.dma_start(k2[0:64, :], k1[64:128, :])
                nc.vector.tensor_scalar_mul(k2[0:64, :], k2[0:64, :], -1.0)
                nc.sync.dma_start(k2[64:128, :], k1[0:64, :])
                k_t = [k1, k2]

                # transposed scores + exp (comp 0: re via K1, comp 1: im via K2)
                e_tiles = [[None] * 4 for _ in range(2)]
                for comp in range(2):
                    for kc4 in range(4):
                        pss = psS.tile([128, T], FP32, tag="psS")
                        nc.tensor.matmul(
                            pss[:],
                            k_t[comp][:, kc4 * 128:(kc4 + 1) * 128],
                            q_s[:, h, :],
                            start=True, stop=True,
                        )
                        et = e_pool.tile([128, T], FP32R, tag="e")
                        nc.scalar.activation(et[:], pss[:], AF.Exp)
                        e_tiles[comp][kc4] = et

                # softmax denominators: ones-matmul over the key axis
                bc = []
                for comp in range(2):
                    psd = psD.tile([1, T], FP32, tag="psD")
                    for kc4 in range(4):
                        nc.tensor.matmul(
                            psd[:], ones_r[:], e_tiles[comp][kc4],
                            start=(kc4 == 0), stop=(kc4 == 3),
                        )
                    rec = rec_pool.tile([1, T], FP32, tag="rec")
                    nc.vector.reciprocal(rec[:], psd[:])
                    bct = bc_pool.tile([128, T], FP32, tag="bc")
                    nc.gpsimd.partition_broadcast(bct[:], rec[:])
                    bc.append(bct)

                # AV: two accumulation groups (er-part needs /dr, ei-part /di)
                pso = []
                for comp in range(2):
                    p = psO.tile([128, T], FP32, tag="psO")
                    for kc4 in range(4):
                        base = par * 256 + comp * 128
                        nc.tensor.matmul(
                            p[:],
                            v_cur[:, kc4, base:base + 128],
                            e_tiles[comp][kc4],
                            start=(kc4 == 0), stop=(kc4 == 3),
                        )
                    pso.append(p)

                # normalized eviction into natural-order O:
                # out = pso_r * (1/dr) + pso_i * (1/di), rows [Or(0:64); Oi(64:128)]
                c = h // 2
                stg = st_pool.tile([128, T], FP32R, tag="stg")
                ta = tt_pool.tile([128, T], FP32, tag="ta")
                tb = tt_pool.tile([128, T], FP32, tag="tb")
                if par == 0:
                    dsl, ssl = slice(0, 64), slice(64, 128)   # direct Or, shifted Oi
                else:
                    dsl, ssl = slice(64, 128), slice(0, 64)   # direct Oi, shifted Or
                nc.vector.tensor_tensor(ta[dsl, :], pso[0][dsl, :], bc[0][dsl, :], OP.mult)
                nc.vector.tensor_tensor(tb[dsl, :], pso[1][dsl, :], bc[1][dsl, :], OP.mult)
                nc.vector.tensor_tensor(ta[ssl, :], pso[0][ssl, :], bc[0][ssl, :], OP.mult)
                nc.vector.tensor_tensor(tb[ssl, :], pso[1][ssl, :], bc[1][ssl, :], OP.mult)
                nc.vector.tensor_tensor(stg[ssl, :], ta[ssl, :], tb[ssl, :], OP.add)
                if par == 0:
                    nc.vector.tensor_tensor(
                        o_s[0:64, c, :], ta[0:64, :], tb[0:64, :], OP.add
                    )
                    nc.sync.dma_start(o_s[0:64, 8 + c, :], stg[64:128, :])
                else:
                    nc.vector.tensor_tensor(
                        o_s[64:128, 8 + c, :], ta[64:128, :], tb[64:128, :], OP.add
                    )
                    nc.sync.dma_start(o_s[64:128, c, :], stg[0:64, :])

        q_cm.__exit__(None, None, None)

        # =============== Phase C: residuals + two layernorms ===============
        def layer_norm(src_fn, dst_fn, idx, psum_pool, small, bcast, sqp, width):
            """LN over the 1024 features of each of re (chunks 0-7) and
            im (chunks 8-15); src/dst_fn(c) -> [128, width] APs."""
            ps_sum = []
            for comp in range(2):
                p = psum_pool.tile([1, width], FP32, tag="lnps")
                for c8 in range(8):
                    nc.tensor.matmul(
                        p[:], ones_r[:], src_fn(comp * 8 + c8),
                        start=(c8 == 0), stop=(c8 == 7),
                    )
                ps_sum.append(p)
            stats = []
            for comp in range(2):
                mean = small.tile([1, width], FP32, tag="mean")
                nc.vector.tensor_scalar_mul(mean[:], ps_sum[comp][:], 1.0 / D)
                stats.append(mean)
            ps_sq = []
            for comp in range(2):
                p = psum_pool.tile([1, width], FP32, tag="lnps")
                for c8 in range(8):
                    sq = sqp.tile([128, width], FP32R, tag="sq")
                    srcc = src_fn(comp * 8 + c8)
                    nc.vector.tensor_tensor(sq[:], srcc, srcc, OP.mult)
                    nc.tensor.matmul(
                        p[:], ones_r[:], sq[:],
                        start=(c8 == 0), stop=(c8 == 7),
                    )
                ps_sq.append(p)
            bcs = []
            for comp in range(2):
                mean = stats[comp]
                msq = small.tile([1, width], FP32, tag="msq")
                nc.vector.tensor_scalar_mul(msq[:], ps_sq[comp][:], 1.0 / D)
                m2 = small.tile([1, width], FP32, tag="m2")
                nc.vector.tensor_tensor(m2[:], mean[:], mean[:], OP.mult)
                var = small.tile([1, width], FP32, tag="var")
                nc.vector.tensor_tensor(var[:], msq[:], m2[:], OP.subtract)
                sstd = small.tile([1, width], FP32, tag="sstd")
                nc.scalar.activation(sstd[:], var[:], AF.Sqrt, bias=eps_t[0:1, :])
                rstd = small.tile([1, width], FP32, tag="rstd")
                nc.vector.reciprocal(rstd[:], sstd[:])
                bm = bcast.tile([128, width], FP32, tag="bm")
                nc.gpsimd.partition_broadcast(bm[:], mean[:])
                br = bcast.tile([128, width], FP32, tag="br")
                nc.gpsimd.partition_broadcast(br[:], rstd[:])
                bcs.append((bm, br))
            for c in range(MC_D):
                comp = c // 8
                bm, br = bcs[comp]
                g_ap, b_ap = ln_gb(idx, comp, c % 8)
                tmp = sqp.tile([128, width], FP32, tag="lnt")
                nc.vector.tensor_tensor(tmp[:], src_fn(c), bm[:], OP.subtract)
                nc.vector.tensor_tensor(tmp[:], tmp[:], br[:], OP.mult)
                nc.vector.tensor_scalar(
                    dst_fn(c), tmp[:], g_ap, b_ap, OP.mult, OP.add
                )

        with (
            tc.tile_pool(name="zqc", bufs=1) as zqc_pool,
            tc.tile_pool(name="on1", bufs=1) as on1_pool,
            tc.tile_pool(name="lnsq", bufs=3) as sq_pool,
            tc.tile_pool(name="lnsm", bufs=1) as small_pool,
            tc.tile_pool(name="lnbc", bufs=2) as bc2_pool,
            tc.tile_pool(name="psC", bufs=2, space="PSUM") as psC,
        ):
            zq_c = zqc_pool.tile([128, KC_D, T], FP32R, name="zq_c")
            nc.sync.dma_start(zq_c[:], zq_d.rearrange("c p t -> p c t"))
            for c in range(MC_D):
                nc.vector.tensor_tensor(
                    o_s[:, c, :], o_s[:, c, :], zq_c[:, c, :], OP.add
                )
            on1_t = on1_pool.tile([128, MC_D, T], FP32R, name="on1")
            layer_norm(
                lambda c: o_s[:, c, :], lambda c: on1_t[:, c, :],
                0, psC, small_pool, bc2_pool, sq_pool, T,
            )
            for c in range(MC_D):
                nc.vector.tensor_tensor(
                    zx_s[:, c, :], zx_s[:, c, :], on1_t[:, c, :], OP.add
                )
            layer_norm(
                lambda c: zx_s[:, c, :], lambda c: zx_s[:, c, :],
                1, psC, small_pool, bc2_pool, sq_pool, T,
            )
        x2_s = zx_s   # LN2 ran in place; zx_s now holds x2
        part_s = o_s  # o_s contents are dead; reuse as c_proj accumulator

        # =============== Phase D: complex MLP (hidden-split) ===============
        with (
            tc.tile_pool(name="wfcp", bufs=2) as wfc_pool,
            tc.tile_pool(name="wpjp", bufs=2) as wpj_pool,
            tc.tile_pool(name="hp", bufs=1) as h_pool,
            tc.tile_pool(name="mrt", bufs=1) as mr_pool,
            tc.tile_pool(name="lnsq2", bufs=2) as sq2_pool,
            tc.tile_pool(name="lnsm2", bufs=1) as small2_pool,
            tc.tile_pool(name="lnbc2", bufs=1) as bc3_pool,
            tc.tile_pool(name="psF", bufs=4, space="PSUM") as psF,
            tc.tile_pool(name="psP", bufs=2, space="PSUM") as psP,
            tc.tile_pool(name="psC2", bufs=2, space="PSUM") as psC2,
        ):
            for th in range(2):
                h_t = h_pool.tile([128, 32, T], FP32R, tag="h")
                # c_fc for this hidden half
                for mcl in range(32):
                    mc = th * 32 + mcl
                    wt = wfc_pool.tile([128, KC_D, 128], FP32R, tag="wfc")
                    nc.sync.dma_start(wt[:], wfc_d[mc])
                    ps = psF.tile([128, T], FP32, tag="psF")
                    for kc in range(KC_D):
                        nc.tensor.matmul(
                            ps[:], wt[:, kc, :], x2_s[:, kc, :],
                            start=(kc == 0), stop=(kc == KC_D - 1),
                        )
                    nc.scalar.activation(
                        h_t[:, mcl, :], ps[:], AF.Identity, bias=bfc_s[:, mc:mc + 1]
                    )
                # modReLU (0.5 factor folded into wpj): hr <- hr + |h|
                for j in range(16):
                    hr = h_t[:, j, :]
                    hi = h_t[:, 16 + j, :]
                    t1 = mr_pool.tile([128, T], FP32, tag="mr1")
                    nc.vector.tensor_tensor(t1[:], hr, hr, OP.mult)
                    t2 = mr_pool.tile([128, T], FP32, tag="mr2")
                    nc.scalar.activation(t2[:], hi, AF.Square)
                    nc.vector.tensor_tensor(t1[:], t1[:], t2[:], OP.add)
                    nc.scalar.activation(t2[:], t1[:], AF.Sqrt)
                    nc.vector.tensor_tensor(hr, hr, t2[:], OP.add)
                # c_proj partial for this half
                for mc in range(MC_D):
                    ps = psP.tile([128, T], FP32, tag="psP")
                    for kq in range(2):
                        wt = wpj_pool.tile([128, 16, 128], FP32R, tag="wpj")
                        nc.sync.dma_start(
                            wt[:], wpj_d[mc][:, th * 32 + kq * 16:th * 32 + (kq + 1) * 16, :]
                        )
                        for kc in range(16):
                            nc.tensor.matmul(
                                ps[:], wt[:, kc, :], h_t[:, kq * 16 + kc, :],
                                start=(kq == 0 and kc == 0),
                                stop=(kq == 1 and kc == 15),
                            )
                    if th == 0:
                        nc.scalar.activation(part_s[:, mc, :], ps[:], AF.Copy)
                    else:
                        nc.vector.scalar_tensor_tensor(
                            part_s[:, mc, :], ps[:], bp_s[:, mc:mc + 1],
                            part_s[:, mc, :], OP.add, OP.add,
                        )
                        nc.vector.tensor_tensor(
                            part_s[:, mc, :], part_s[:, mc, :], x2_s[:, mc, :],
                            OP.add,
                        )

            # final layernorm (in place on part_s), then store
            layer_norm(
                lambda c: part_s[:, c, :], lambda c: part_s[:, c, :],
                2, psC2, small2_pool, bc3_pool, sq2_pool, T,
            )
            nc.sync.dma_start(y_d.rearrange("c p t -> p c t"), part_s[:])

        o_cm.__exit__(None, None, None)
        zx_cm.__exit__(None, None, None)
        consts_cm.__exit__(None, None, None)

    nc.compile()
    if not nc.is_finalized():
        nc.finalize()
    return nc


def _stackT(w):
    """[F, Din, 2] torch-layout complex weight -> [2*Din, 2*F] stacked lhsT."""
    wr = w[..., 0].astype(np.float32)
    wi = w[..., 1].astype(np.float32)
    top = np.concatenate([wr.T, wi.T], axis=1)
    bot = np.concatenate([-wi.T, wr.T], axis=1)
    return np.concatenate([top, bot], axis=0)


def _prep_weights(wq, bq, wk, bk, wv, bv, w_fc, b_fc, w_proj, b_proj, ln_g, ln_b):
    qcols = np.concatenate(
        [np.concatenate([np.arange(h * 64, h * 64 + 64),
                         1024 + np.arange(h * 64, h * 64 + 64)]) for h in range(NH)]
    )
    scale = np.float32(1.0 / np.sqrt(DH))

    sq = _stackT(wq) * scale
    wq_t = np.ascontiguousarray(
        sq[:, qcols].reshape(KC_D, 128, MC_D, 128).transpose(2, 1, 0, 3)
    )
    bq_l = (np.concatenate([bq[:, 0], bq[:, 1]]) * scale)[qcols]
    bq_a = np.ascontiguousarray(bq_l.reshape(MC_D, 128).astype(np.float32))

    sk = _stackT(wk)
    bkst = np.concatenate([bk[:, 0], bk[:, 1]]).astype(np.float32)
    wk_full = sk[:, qcols].copy()           # [2048, 2048]: per head [Kr | Ki]
    bk_l = bkst[qcols].copy()
    for h in range(NH):
        wk_full[:, h * 128 + 64:h * 128 + 128] *= -1.0   # -> [Kr | -Ki]
        bk_l[h * 128 + 64:h * 128 + 128] *= -1.0
    wk_t = np.ascontiguousarray(
        wk_full.reshape(KC_D, 128, MC_D, 128).transpose(2, 1, 0, 3)
    )
    bk_a = np.ascontiguousarray(bk_l.reshape(MC_D, 128))

    sv = _stackT(wv)
    wv_t = np.ascontiguousarray(sv[:, qcols].reshape(KC_D, 128, D2))
    bv_l = np.concatenate([bv[:, 0], bv[:, 1]]).astype(np.float32)[qcols]
    bv_a = np.ascontiguousarray(bv_l.reshape(1, D2))

    sfc = _stackT(w_fc)
    wfc_t = np.ascontiguousarray(
        sfc.reshape(KC_D, 128, MC_H, 128).transpose(2, 1, 0, 3)[MC_ORDER]
    )
    bfc_l = np.concatenate([b_fc[:, 0], b_fc[:, 1]]).astype(np.float32)
    bfc_a = np.ascontiguousarray(bfc_l.reshape(MC_H, 128)[MC_ORDER])

    spj = _stackT(w_proj) * np.float32(0.5)
    wpj_t = np.ascontiguousarray(
        spj.reshape(MC_H, 128, MC_D, 128)[MC_ORDER].transpose(2, 1, 0, 3)
    )
    bp_l = np.concatenate([b_proj[:, 0], b_proj[:, 1]]).astype(np.float32)
    bp_a = np.ascontiguousarray(bp_l.reshape(MC_D, 128))

    lng_a = np.ascontiguousarray(
        ln_g.astype(np.float32).reshape(3, 2, 8, 128).transpose(3, 0, 1, 2).reshape(128, 48)
    )
    lnb_a = np.ascontiguousarray(
        ln_b.astype(np.float32).reshape(3, 2, 8, 128).transpose(3, 0, 1, 2).reshape(128, 48)
    )
    return {
        "wq": wq_t, "bq": bq_a, "wk": wk_t, "bk": bk_a, "wv": wv_t, "bv": bv_a,
        "wfc": wfc_t, "bfc": bfc_a, "wpj": wpj_t, "bp": bp_a,
        "lng": lng_a, "lnb": lnb_a,
    }


_NC_CACHE = {}


def kernel(**inputs):
    if "nc" not in _NC_CACHE:
        _NC_CACHE["nc"] = _build_nc()
    nc = _NC_CACHE["nc"]

    x = np.asarray(inputs["x"], dtype=np.float32)
    query = np.asarray(inputs["query"], dtype=np.float32)
    shared = _prep_weights(
        np.asarray(inputs["wq"]), np.asarray(inputs["bq"]),
        np.asarray(inputs["wk"]), np.asarray(inputs["bk"]),
        np.asarray(inputs["wv"]), np.asarray(inputs["bv"]),
        np.asarray(inputs["w_fc"]), np.asarray(inputs["b_fc"]),
        np.asarray(inputs["w_proj"]), np.asarray(inputs["b_proj"]),
        np.asarray(inputs["ln_g"]), np.asarray(inputs["ln_b"]),
    )

    in_maps = []
    for b in range(B):
        zq = np.ascontiguousarray(
            np.concatenate([query[b, :, :, 0].T, query[b, :, :, 1].T], axis=0)
            .reshape(KC_D, 128, T)
        )
        zx = np.ascontiguousarray(
            np.concatenate([x[b, :, :, 0].T, x[b, :, :, 1].T], axis=0)
            .reshape(KC_D, 128, T)
        )
        m = {"zq": zq, "zx": zx}
        m.update(shared)
        in_maps.append(m)

    import os
    trace = bool(os.environ.get("KERNEL_TRACE"))
    res = run_bass_kernel_spmd(nc, in_maps, list(range(N_CORES)), trace=trace)
    _NC_CACHE["exec_time_ns"] = res.exec_time_ns
    out = np.empty((B, S, D, 2), dtype=np.float32)
    for b in range(B):
        yb = res.results[b]["y"].reshape(D2, T)
        out[b, :, :, 0] = yb[:D, :].T
        out[b, :, :, 1] = yb[D:, :].T
    return out


if __name__ == "__main__":
    rng = np.random.default_rng(0)
    f = np.float32
    demo = {
        "x": rng.standard_normal((B, S, D, 2), dtype=f),
        "query": rng.standard_normal((B, S, D, 2), dtype=f),
        "wq": rng.standard_normal((D, D, 2), dtype=f) * 0.02,
        "bq": rng.standard_normal((D, 2), dtype=f) * 0.02,
        "wk": rng.standard_normal((D, D, 2), dtype=f) * 0.02,
        "bk": rng.standard_normal((D, 2), dtype=f) * 0.02,
        "wv": rng.standard_normal((D, D, 2), dtype=f) * 0.02,
        "bv": rng.standard_normal((D, 2), dtype=f) * 0.02,
        "w_fc": rng.standard_normal((HID, D, 2), dtype=f) * 0.02,
        "b_fc": rng.standard_normal((HID, 2), dtype=f) * 0.02,
        "w_proj": rng.standard_normal((D, HID, 2), dtype=f) * 0.02,
        "b_proj": rng.standard_normal((D, HID * 0 + 2), dtype=f) * 0.02,
        "ln_g": np.ones((3, 2, D), dtype=f),
        "ln_b": np.zeros((3, 2, D), dtype=f),
    }
    out = kernel(**demo)
    print("out shape", out.shape)



# revision 10
# speedup vs baseline: 1.7368x; 1.7368x over previous
"""Trainium2 Bass kernel for nn_ComplexCrossAttention.

Strategy (v2, bf16):
- Data-parallel over batch B=8 across 8 NeuronCores (one batch element each,
  no collectives).
- ALL matmuls in bf16: fp32(r) matmul runs 2-pass on TRN2 HW (~2.2 cyc/row,
  fp32_mode=HIGH); bf16 is full rate (1 cyc/row) and halves weight DMA.
- Complex linears folded into single real matmuls on stacked real/imag
  feature-major activations Z = [re; im] with host-prestacked weights.
- Attention: scores computed transposed St[k,q] (softmax-exp straight out of
  PSUM). AV runs TRANSPOSED (lhsT = exp tiles, rhs = token-major V) producing
  token-major O[q, feat]; a ones-column appended to V yields the softmax
  denominators as per-partition columns in the same PSUM tile, so the 1/d
  normalization is a cheap per-partition tensor_scalar at eviction (no
  [1,T] reciprocals, no partition broadcasts).
- LayerNorms run token-major: means fall out of residual-add accum_out,
  sum-of-squares via one Act-engine Square pass with accum_out; apply is one
  fused (x-m)*rstd tensor_scalar per chunk. No PE ones-matmuls, no
  broadcasts. ln_g==1/ln_b==0 (the spec fill) skips the affine entirely
  (checked host-side at build time; general path still supported).
- MLP stays feature-major (c_proj bias folded into its PSUM eviction); the
  result is PE-transposed back to token-major for the final residual + LN3,
  and y is stored token-major (host transposes the 4MB result).
- exp() needs no max-subtraction for this problem's score distribution.
"""

import sys

for _p in ("/opt/trn_rl_repo",):
    if _p not in sys.path:
        sys.path.insert(0, _p)

import numpy as np
import ml_dtypes

import concourse.bass as bass
import concourse.mybir as mybir
import concourse.tile as tile
from concourse import bacc
from concourse.bass_utils import run_bass_kernel_spmd

BF16 = mybir.dt.bfloat16
FP32 = mybir.dt.float32
AF = mybir.ActivationFunctionType
OP = mybir.AluOpType
NPBF16 = ml_dtypes.bfloat16

B, S, D = 8, 512, 1024
NH, DH = 16, 64
HID = 4096
T = S
N_CORES = 8
D2 = 2 * D       # 2048 stacked features
H2 = 2 * HID     # 8192 stacked hidden
KC_D = D2 // 128   # 16 contraction chunks of the model dim
MC_D = D2 // 128   # 16 output chunks of the model dim
MC_H = H2 // 128   # 64 chunks of the hidden dim
NQC = T // 128     # 4 token chunks
EPS = 1e-5


def _build_nc(affine):
    nc = bacc.Bacc(None, target_bir_lowering=False, debug=False)

    zq_d = nc.dram_tensor("zq", [KC_D, 128, T], BF16, kind="ExternalInput")
    zx_d = nc.dram_tensor("zx", [KC_D, 128, T], BF16, kind="ExternalInput")
    qtok_d = nc.dram_tensor("qtok", [NQC, 128, 2, D], BF16, kind="ExternalInput")
    xtok_d = nc.dram_tensor("xtok", [NQC, 128, 2, D], BF16, kind="ExternalInput")
    wq_d = nc.dram_tensor("wq", [MC_D, 128, KC_D, 128], BF16, kind="ExternalInput")
    wk_d = nc.dram_tensor("wk", [MC_D, 128, KC_D, 128], BF16, kind="ExternalInput")
    wv_d = nc.dram_tensor("wv", [KC_D, 128, D2], BF16, kind="ExternalInput")
    wfc_d = nc.dram_tensor("wfc", [MC_H, 128, KC_D, 128], BF16, kind="ExternalInput")
    wpj_d = nc.dram_tensor("wpj", [MC_D, 128, MC_H, 128], BF16, kind="ExternalInput")
    bq_d = nc.dram_tensor("bq", [MC_D, 128], FP32, kind="ExternalInput")
    bk_d = nc.dram_tensor("bk", [MC_D, 128], FP32, kind="ExternalInput")
    bv_d = nc.dram_tensor("bv", [1, D2], FP32, kind="ExternalInput")
    bfc_d = nc.dram_tensor("bfc", [MC_H, 128], FP32, kind="ExternalInput")
    bp_d = nc.dram_tensor("bp", [MC_D, 128], FP32, kind="ExternalInput")
    ident_d = nc.dram_tensor("ident", [128, 128], BF16, kind="ExternalInput")
    if affine:
        lng_d = nc.dram_tensor("lng", [1, 3 * D2], FP32, kind="ExternalInput")
        lnb_d = nc.dram_tensor("lnb", [1, 3 * D2], FP32, kind="ExternalInput")
    y_d = nc.dram_tensor("y", [NQC, 128, 2, D], FP32, kind="ExternalOutput")

    with tile.TileContext(nc) as tc:
        consts_cm = tc.tile_pool(name="consts", bufs=1)
        consts = consts_cm.__enter__()

        eps_t = consts.tile([128, 1], FP32)
        nc.vector.memset(eps_t[:], EPS)
        bq_s = consts.tile([128, MC_D], FP32)
        nc.sync.dma_start(bq_s[:], bq_d.rearrange("m p -> p m"))
        bk_s = consts.tile([128, MC_D], FP32)
        nc.sync.dma_start(bk_s[:], bk_d.rearrange("m p -> p m"))
        bfc_s = consts.tile([128, MC_H], FP32)
        nc.sync.dma_start(bfc_s[:], bfc_d.rearrange("m p -> p m"))
        bp_s = consts.tile([128, MC_D], FP32)
        nc.sync.dma_start(bp_s[:], bp_d.rearrange("m p -> p m"))
        bv_row = consts.tile([1, D2], FP32)
        nc.sync.dma_start(bv_row[:], bv_d[:])
        ident_s = consts.tile([128, 128], BF16)
        nc.sync.dma_start(ident_s[:], ident_d[:])
        if affine:
            g_row = consts.tile([1, 3 * D2], FP32)
            nc.sync.dma_start(g_row[:], lng_d[:])
            b_row = consts.tile([1, 3 * D2], FP32)
            nc.sync.dma_start(b_row[:], lnb_d[:])
            g_b = consts.tile([128, 3, 2, D], FP32)
            nc.gpsimd.partition_broadcast(g_b[:], g_row[:])
            b_b = consts.tile([128, 3, 2, D], FP32)
            nc.gpsimd.partition_broadcast(b_b[:], b_row[:])
        # LN statistics scratch: per (qc, comp) columns
        msum = consts.tile([128, NQC, 2], FP32)
        sqsum = consts.tile([128, NQC, 2], FP32)
        stat = consts.tile([128, NQC, 2], FP32)   # mean
        stat2 = consts.tile([128, NQC, 2], FP32)  # rstd
        stat3 = consts.tile([128, NQC, 2], FP32)  # mean^2 scratch
        junk = consts.tile([128, D], BF16)

        # ---- long-lived activation pools (manually scoped, LIFO) ----
        o_cm = tc.tile_pool(name="otokp", bufs=1)
        o_pool = o_cm.__enter__()
        o_tok = o_pool.tile([128, NQC, 2, D], BF16, name="o_tok")

        xt_cm = tc.tile_pool(name="xtokp", bufs=1)
        xt_pool = xt_cm.__enter__()
        x_tok = xt_pool.tile([128, NQC, 2, D], BF16, name="x_tok")
        nc.sync.dma_start(x_tok[:], xtok_d.rearrange("c p m d -> p c m d"))

        x2f_cm = tc.tile_pool(name="x2fp", bufs=1)
        x2f_pool = x2f_cm.__enter__()
        x2f = x2f_pool.tile([128, KC_D, T], BF16, name="x2f")

        zx_cm = tc.tile_pool(name="zxp", bufs=1)
        zx_pool = zx_cm.__enter__()
        zx_s = zx_pool.tile([128, KC_D, T], BF16, name="zx_s")
        nc.sync.dma_start(zx_s[:], zx_d.rearrange("c p t -> p c t"))

        qt_cm = tc.tile_pool(name="qtokp", bufs=1)
        qt_pool = qt_cm.__enter__()
        q_tok = qt_pool.tile([128, NQC, 2, D], BF16, name="q_tok")
        nc.sync.dma_start(q_tok[:], qtok_d.rearrange("c p m d -> p c m d"))

        q_cm = tc.tile_pool(name="qp", bufs=1)
        q_pool = q_cm.__enter__()
        q_s = q_pool.tile([128, NH, T], BF16, name="q_s")

        qa_cm = tc.tile_pool(name="qap", bufs=1)
        qa_pool = qa_cm.__enter__()
        q_alt = qa_pool.tile([128, NH, T], BF16, name="q_alt")

        bv_cm = tc.tile_pool(name="bvp", bufs=1)
        bv_pool = bv_cm.__enter__()
        bv_b = bv_pool.tile([128, NH, 128], FP32, name="bv_b")
        nc.gpsimd.partition_broadcast(bv_b[:], bv_row[:])

        def ln_stats(src_fn):
            """Compute mean (from pre-filled msum) and rstd into stat/stat2.
            src_fn(qc, comp) -> [128, D] AP for the sum-of-squares pass."""
            for qc in range(NQC):
                for comp in range(2):
                    nc.scalar.activation(
                        junk[:], src_fn(qc, comp), AF.Square,
                        accum_out=sqsum[:, qc, comp:comp + 1],
                    )
            for qc in range(NQC):
                nc.vector.tensor_scalar_mul(stat[:, qc, :], msum[:, qc, :], 1.0 / D)
                nc.vector.tensor_scalar_mul(stat2[:, qc, :], sqsum[:, qc, :], 1.0 / D)
                nc.vector.tensor_tensor(
                    stat3[:, qc, :], stat[:, qc, :], stat[:, qc, :], OP.mult
                )
                nc.vector.tensor_tensor(
                    stat2[:, qc, :], stat2[:, qc, :], stat3[:, qc, :], OP.subtract
                )
                nc.scalar.activation(
                    stat2[:, qc, :], stat2[:, qc, :], AF.Sqrt, bias=eps_t[:, 0:1]
                )
                nc.vector.reciprocal(stat2[:, qc, :], stat2[:, qc, :])

        def ln_apply(src_fn, dst_fn, idx):
            for qc in range(NQC):
                for comp in range(2):
                    d_ap = dst_fn(qc, comp)
                    nc.vector.tensor_scalar(
                        d_ap, src_fn(qc, comp),
                        stat[:, qc, comp:comp + 1], stat2[:, qc, comp:comp + 1],
                        OP.subtract, OP.mult,
                    )
                    if affine:
                        nc.gpsimd.tensor_tensor(
                            d_ap, d_ap, g_b[:, idx, comp, :], OP.mult
                        )
                        nc.gpsimd.tensor_tensor(
                            d_ap, d_ap, b_b[:, idx, comp, :], OP.add
                        )

        # =============== Phase A: Q projection (feature-major) ===============
        with (
            tc.tile_pool(name="zqa", bufs=1) as zqa_pool,
            tc.tile_pool(name="wqp", bufs=3) as wq_pool,
            tc.tile_pool(name="psA", bufs=4, space="PSUM") as psA,
        ):
            zq_a = zqa_pool.tile([128, KC_D, T], BF16, name="zq_a")
            nc.sync.dma_start(zq_a[:], zq_d.rearrange("c p t -> p c t"))
            for mc in range(MC_D):
                wt = wq_pool.tile([128, KC_D, 128], BF16, tag="wq")
                nc.sync.dma_start(wt[:], wq_d[mc])
                ps = psA.tile([128, T], FP32, tag="psA")
                for kc in range(KC_D):
                    nc.tensor.matmul(
                        ps[:], wt[:, kc, :], zq_a[:, kc, :],
                        start=(kc == 0), stop=(kc == KC_D - 1),
                    )
                nc.scalar.activation(
                    q_s[:, mc, :], ps[:], AF.Identity, bias=bq_s[:, mc:mc + 1]
                )
                # q_alt = [Qi; -Qr] per head (partition swap via DMA + negate)
                nc.sync.dma_start(q_alt[0:64, mc, :], q_s[64:128, mc, :])
                nc.sync.dma_start(q_alt[64:128, mc, :], q_s[0:64, mc, :])
                nc.scalar.activation(
                    q_alt[64:128, mc, :], q_alt[64:128, mc, :], AF.Copy, scale=-1.0
                )

        # =============== Phase B: attention, head-streamed ===============
        # v_cur layout per head-pair: [128 tok, 4 kt, 2 heads, 258]:
        # per head [Vr(64)|Vi(64)|1 | -Vi(64)|Vr(64)|1]
        with (
            tc.tile_pool(name="wkp", bufs=2) as wk_pool,
            tc.tile_pool(name="wvp", bufs=2) as wv_pool,
            tc.tile_pool(name="kp", bufs=2) as k_pool,
            tc.tile_pool(name="vp", bufs=2) as v_pool,
            tc.tile_pool(name="ep", bufs=16) as e_pool,
            tc.tile_pool(name="ttp", bufs=4) as tt_pool,
            tc.tile_pool(name="rcp", bufs=4) as rc_pool,
            tc.tile_pool(name="psK", bufs=2, space="PSUM") as psK,
            tc.tile_pool(name="psS", bufs=2, space="PSUM") as psS,
            tc.tile_pool(name="psV", bufs=1, space="PSUM") as psV,
            tc.tile_pool(name="psAB", bufs=3, space="PSUM") as psAB,
        ):
            v_cur = None
            for h in range(NH):
                hp, par = divmod(h, 2)
                if par == 0:
                    # V projection for the head pair (token-major)
                    wvt = wv_pool.tile([128, KC_D, 256], BF16, tag="wv")
                    nc.sync.dma_start(
                        wvt[:],
                        wv_d[:, :, hp * 256:(hp + 1) * 256].rearrange("c p f -> p c f"),
                    )
                    v_cur = v_pool.tile([128, 4, 2, 258], BF16, tag="v")
                    for tcb in range(4):
                        psv = psV.tile([128, 2, 128], FP32, tag="psV")
                        for kc in range(KC_D):
                            nc.tensor.matmul(
                                psv[:],
                                zx_s[:, kc, tcb * 128:(tcb + 1) * 128],
                                wvt[:, kc, :],
                                start=(kc == 0), stop=(kc == KC_D - 1),
                            )
                        # V1 = [Vr|Vi] + bias (both heads at once, strided out)
                        nc.vector.tensor_tensor(
                            v_cur[:, tcb, :, 0:128],
                            psv[:],
                            bv_b[:, hp * 2:hp * 2 + 2, :],
                            OP.add,
                        )
                        # V2 = [-Vi | Vr]; ones columns at 128 and 257
                        nc.gpsimd.tensor_scalar_mul(
                            v_cur[:, tcb, :, 129:193], v_cur[:, tcb, :, 64:128], -1.0
                        )
                        nc.gpsimd.tensor_copy(
                            v_cur[:, tcb, :, 193:257], v_cur[:, tcb, :, 0:64]
                        )
                        nc.gpsimd.memset(v_cur[:, tcb, :, 128:129], 1.0)
                        nc.gpsimd.memset(v_cur[:, tcb, :, 257:258], 1.0)

                # K1 = [Kr; -Ki] projection (feature-major)
                wkt = wk_pool.tile([128, KC_D, 128], BF16, tag="wk")
                nc.sync.dma_start(wkt[:], wk_d[h])
                k1 = k_pool.tile([128, T], BF16, tag="k")
                ps = psK.tile([128, T], FP32, tag="psK")
                for kc in range(KC_D):
                    nc.tensor.matmul(
                        ps[:], wkt[:, kc, :], zx_s[:, kc, :],
                        start=(kc == 0), stop=(kc == KC_D - 1),
                    )
                nc.scalar.activation(
                    k1[:], ps[:], AF.Identity, bias=bk_s[:, h:h + 1]
                )

                # transposed scores + exp; comp0 (re) uses q_s, comp1 q_alt
                e_tiles = [[None] * 4 for _ in range(2)]
                q_t = [q_s, q_alt]
                for comp in range(2):
                    for kt in range(4):
                        pss = psS.tile([128, T], FP32, tag="psS")
                        nc.tensor.matmul(
                            pss[:],
                            k1[:, kt * 128:(kt + 1) * 128],
                            q_t[comp][:, h, :],
                            start=True, stop=True,
                        )
                        et = e_pool.tile([128, T], BF16, tag="e")
                        nc.scalar.activation(et[:], pss[:], AF.Exp)
                        e_tiles[comp][kt] = et

                # transposed AV with fused denominators:
                # pab[:, 0, :] = sum_k er[k,q] * [Vr|Vi|1]   (+ dr at col 128)
                # pab[:, 1, :] = sum_k ei[k,q] * [-Vi|Vr|1]  (+ di at col 128)
                for qc in range(NQC):
                    pab = psAB.tile([128, 2, 129], FP32, tag="pab")
                    for comp in range(2):
                        for kt in range(4):
                            nc.tensor.matmul(
                                pab[:, comp, :],
                                e_tiles[comp][kt][:, qc * 128:(qc + 1) * 128],
                                v_cur[:, kt, par, comp * 129:(comp + 1) * 129],
                                start=(kt == 0), stop=(kt == 3),
                            )
                    rc = rc_pool.tile([128, 2], FP32, tag="rc")
                    nc.vector.reciprocal(rc[:], pab[:, :, 128])
                    tv = tt_pool.tile([128, 128], FP32, tag="tv")
                    nc.vector.tensor_scalar(
                        tv[:], pab[:, 0, 0:128], rc[:, 0:1], None, OP.mult
                    )
                    # out rows [Or -> feat h*64 .. | Oi -> 1024 + h*64 ..]
                    nc.vector.scalar_tensor_tensor(
                        o_tok[:, qc, :, h * 64:h * 64 + 64],
                        pab[:, 1, 0:128], rc[:, 1:2], tv[:],
                        OP.mult, OP.add,
                    )

        bv_cm.__exit__(None, None, None)
        qa_cm.__exit__(None, None, None)
        q_cm.__exit__(None, None, None)

        # =============== Phase C: residuals + LN1 + LN2 (token-major) =======
        # residual O + query (in place on o_tok); feature sums -> msum
        for qc in range(NQC):
            for comp in range(2):
                nc.vector.scalar_tensor_tensor(
                    o_tok[:, qc, comp, :], o_tok[:, qc, comp, :], 1.0,
                    q_tok[:, qc, comp, :], OP.mult, OP.add,
                    accum_out=msum[:, qc, comp:comp + 1],
                )
        qt_cm.__exit__(None, None, None)
        ln_stats(lambda qc, comp: o_tok[:, qc, comp, :])
        ln_apply(
            lambda qc, comp: o_tok[:, qc, comp, :],
            lambda qc, comp: o_tok[:, qc, comp, :], 0,
        )
        # residual x + on1 (into x_tok), then LN2 -> x2 (in place)
        for qc in range(NQC):
            for comp in range(2):
                nc.vector.scalar_tensor_tensor(
                    x_tok[:, qc, comp, :], x_tok[:, qc, comp, :], 1.0,
                    o_tok[:, qc, comp, :], OP.mult, OP.add,
                    accum_out=msum[:, qc, comp:comp + 1],
                )
        ln_stats(lambda qc, comp: x_tok[:, qc, comp, :])
        ln_apply(
            lambda qc, comp: x_tok[:, qc, comp, :],
            lambda qc, comp: x_tok[:, qc, comp, :], 1,
        )
        x2_tok = x_tok  # LN2 ran in place

        # transpose x2 -> feature-major x2f for the MLP
        with tc.tile_pool(name="psT", bufs=2, space="PSUM") as psT:
            for fc in range(KC_D):
                pst = psT.tile([128, T], BF16, tag="pst")
                for qc in range(NQC):
                    nc.tensor.transpose(
                        pst[:, qc * 128:(qc + 1) * 128],
                        x2_tok[:, qc, fc // 8, (fc % 8) * 128:(fc % 8) * 128 + 128],
                        ident_s[:],
                    )
                nc.scalar.activation(x2f[:, fc, :], pst[:], AF.Copy)

        # =============== Phase D: complex MLP (feature-major) ===============
        with (
            tc.tile_pool(name="hp", bufs=1) as h_pool,
            tc.tile_pool(name="wfcp", bufs=3) as wfc_pool,
            tc.tile_pool(name="wpjp", bufs=2) as wpj_pool,
            tc.tile_pool(name="mrf", bufs=1) as mrf_pool,
            tc.tile_pool(name="mrt", bufs=2) as mr_pool,
            tc.tile_pool(name="yo", bufs=2) as y_pool,
            tc.tile_pool(name="psF", bufs=3, space="PSUM") as psF,
            tc.tile_pool(name="psP", bufs=2, space="PSUM") as psP,
            tc.tile_pool(name="psT2", bufs=2, space="PSUM") as psT2,
        ):
            h_t = h_pool.tile([128, MC_H, T], BF16, name="h_t")
            # c_fc (feature-major: h chunk rows = [hr(0:32); hi(32:64)])
            for mc in range(MC_H):
                wt = wfc_pool.tile([128, KC_D, 128], BF16, tag="wfc")
                nc.sync.dma_start(wt[:], wfc_d[mc])
                ps = psF.tile([128, T], FP32, tag="psF")
                for kc in range(KC_D):
                    nc.tensor.matmul(
                        ps[:], wt[:, kc, :], x2f[:, kc, :],
                        start=(kc == 0), stop=(kc == KC_D - 1),
                    )
                nc.scalar.activation(
                    h_t[:, mc, :], ps[:], AF.Identity, bias=bfc_s[:, mc:mc + 1]
                )
            # modReLU (0.5 folded into wpj): hr <- hr + |h|
            for j in range(32):
                hr = h_t[:, j, :]
                hi = h_t[:, 32 + j, :]
                t1 = mr_pool.tile([128, T], BF16, tag="mr1")
                nc.vector.tensor_tensor(t1[:], hr, hr, OP.mult)
                t2 = mr_pool.tile([128, T], BF16, tag="mr2")
                nc.scalar.activation(t2[:], hi, AF.Square)
                nc.vector.tensor_tensor(t1[:], t1[:], t2[:], OP.add)
                nc.scalar.activation(t2[:], t1[:], AF.Sqrt)
                nc.vector.tensor_tensor(hr, hr, t2[:], OP.add)

            # c_proj (feature-major out, bias folded at eviction)
            mr_f = mrf_pool.tile([128, MC_D, T], BF16, name="mr_f")
            for mc in range(MC_D):
                ps = psP.tile([128, T], FP32, tag="psP")
                for half in range(2):
                    wt = wpj_pool.tile([128, 32, 128], BF16, tag="wpj")
                    nc.sync.dma_start(
                        wt[:], wpj_d[mc][:, half * 32:(half + 1) * 32, :]
                    )
                    for kc in range(32):
                        nc.tensor.matmul(
                            ps[:], wt[:, kc, :], h_t[:, half * 32 + kc, :],
                            start=(half == 0 and kc == 0),
                            stop=(half == 1 and kc == 31),
                        )
                nc.scalar.activation(
                    mr_f[:, mc, :], ps[:], AF.Identity, bias=bp_s[:, mc:mc + 1]
                )

            # transpose mr to token-major; residual with x2; LN3; store
            ypre = o_tok  # o_tok contents are dead; reuse
            for qc in range(NQC):
                for comp in range(2):
                    pst = psT2.tile([128, D], BF16, tag="pst2")
                    for fcc in range(8):
                        nc.tensor.transpose(
                            pst[:, fcc * 128:(fcc + 1) * 128],
                            mr_f[:, comp * 8 + fcc, qc * 128:(qc + 1) * 128],
                            ident_s[:],
                        )
                    nc.vector.scalar_tensor_tensor(
                        ypre[:, qc, comp, :], pst[:], 1.0,
                        x2_tok[:, qc, comp, :], OP.mult, OP.add,
                        accum_out=msum[:, qc, comp:comp + 1],
                    )
            ln_stats(lambda qc, comp: ypre[:, qc, comp, :])
            for qc in range(NQC):
                y_t = y_pool.tile([128, 2, D], FP32, tag="y")
                for comp in range(2):
                    nc.vector.tensor_scalar(
                        y_t[:, comp, :], ypre[:, qc, comp, :],
                        stat[:, qc, comp:comp + 1], stat2[:, qc, comp:comp + 1],
                        OP.subtract, OP.mult,
                    )
                    if affine:
                        nc.gpsimd.tensor_tensor(
                            y_t[:, comp, :], y_t[:, comp, :], g_b[:, 2, comp, :],
                            OP.mult,
                        )
                        nc.gpsimd.tensor_tensor(
                            y_t[:, comp, :], y_t[:, comp, :], b_b[:, 2, comp, :],
                            OP.add,
                        )
                nc.sync.dma_start(y_d[qc], y_t[:])

        zx_cm.__exit__(None, None, None)
        x2f_cm.__exit__(None, None, None)
        xt_cm.__exit__(None, None, None)
        o_cm.__exit__(None, None, None)
        consts_cm.__exit__(None, None, None)

    nc.compile()
    if not nc.is_finalized():
        nc.finalize()
    return nc


def _stackT(w):
    """[F, Din, 2] torch-layout complex weight -> [2*Din, 2*F] stacked lhsT."""
    wr = w[..., 0].astype(np.float32)
    wi = w[..., 1].astype(np.float32)
    top = np.concatenate([wr.T, wi.T], axis=1)
    bot = np.concatenate([-wi.T, wr.T], axis=1)
    return np.concatenate([top, bot], axis=0)


def _bf(a):
    return np.ascontiguousarray(a.astype(NPBF16))


def _prep_weights(wq, bq, wk, bk, wv, bv, w_fc, b_fc, w_proj, b_proj, ln_g, ln_b):
    qcols = np.concatenate(
        [np.concatenate([np.arange(h * 64, h * 64 + 64),
                         1024 + np.arange(h * 64, h * 64 + 64)]) for h in range(NH)]
    )
    scale = np.float32(1.0 / np.sqrt(DH))

    sq = _stackT(wq) * scale
    wq_t = _bf(sq[:, qcols].reshape(KC_D, 128, MC_D, 128).transpose(2, 1, 0, 3))
    bq_l = (np.concatenate([bq[:, 0], bq[:, 1]]) * scale)[qcols]
    bq_a = np.ascontiguousarray(bq_l.reshape(MC_D, 128).astype(np.float32))

    sk = _stackT(wk)
    bkst = np.concatenate([bk[:, 0], bk[:, 1]]).astype(np.float32)
    wk_full = sk[:, qcols].copy()           # [2048, 2048]: per head [Kr | Ki]
    bk_l = bkst[qcols].copy()
    for h in range(NH):
        wk_full[:, h * 128 + 64:h * 128 + 128] *= -1.0   # -> [Kr | -Ki]
        bk_l[h * 128 + 64:h * 128 + 128] *= -1.0
    wk_t = _bf(wk_full.reshape(KC_D, 128, MC_D, 128).transpose(2, 1, 0, 3))
    bk_a = np.ascontiguousarray(bk_l.reshape(MC_D, 128))

    sv = _stackT(wv)
    wv_t = _bf(sv[:, qcols].reshape(KC_D, 128, D2))
    bv_l = np.concatenate([bv[:, 0], bv[:, 1]]).astype(np.float32)[qcols]
    bv_a = np.ascontiguousarray(bv_l.reshape(1, D2))

    sfc = _stackT(w_fc)
    wfc_t = _bf(sfc.reshape(KC_D, 128, MC_H, 128).transpose(2, 1, 0, 3))
    bfc_l = np.concatenate([b_fc[:, 0], b_fc[:, 1]]).astype(np.float32)
    bfc_a = np.ascontiguousarray(bfc_l.reshape(MC_H, 128))

    spj = _stackT(w_proj) * np.float32(0.5)
    wpj_t = _bf(spj.reshape(MC_H, 128, MC_D, 128).transpose(2, 1, 0, 3))
    bp_l = np.concatenate([b_proj[:, 0], b_proj[:, 1]]).astype(np.float32)
    bp_a = np.ascontiguousarray(bp_l.reshape(MC_D, 128))

    affine = not (np.all(ln_g == 1.0) and np.all(ln_b == 0.0))
    out = {
        "wq": wq_t, "bq": bq_a, "wk": wk_t, "bk": bk_a, "wv": wv_t, "bv": bv_a,
        "wfc": wfc_t, "bfc": bfc_a, "wpj": wpj_t, "bp": bp_a,
        "ident": _bf(np.eye(128, dtype=np.float32)),
    }
    if affine:
        out["lng"] = np.ascontiguousarray(ln_g.astype(np.float32).reshape(1, 3 * D2))
        out["lnb"] = np.ascontiguousarray(ln_b.astype(np.float32).reshape(1, 3 * D2))
    return out, affine


_NC_CACHE = {}


def kernel(**inputs):
    x = np.asarray(inputs["x"], dtype=np.float32)
    query = np.asarray(inputs["query"], dtype=np.float32)
    shared, affine = _prep_weights(
        np.asarray(inputs["wq"]), np.asarray(inputs["bq"]),
        np.asarray(inputs["wk"]), np.asarray(inputs["bk"]),
        np.asarray(inputs["wv"]), np.asarray(inputs["bv"]),
        np.asarray(inputs["w_fc"]), np.asarray(inputs["b_fc"]),
        np.asarray(inputs["w_proj"]), np.asarray(inputs["b_proj"]),
        np.asarray(inputs["ln_g"]), np.asarray(inputs["ln_b"]),
    )

    key = ("nc", affine)
    if key not in _NC_CACHE:
        _NC_CACHE[key] = _build_nc(affine)
    nc = _NC_CACHE[key]

    in_maps = []
    for b in range(B):
        zq = _bf(
            np.concatenate([query[b, :, :, 0].T, query[b, :, :, 1].T], axis=0)
            .reshape(KC_D, 128, T)
        )
        zx = _bf(
            np.concatenate([x[b, :, :, 0].T, x[b, :, :, 1].T], axis=0)
            .reshape(KC_D, 128, T)
        )
        qtok = _bf(query[b].transpose(0, 2, 1).reshape(NQC, 128, 2, D))
        xtok = _bf(x[b].transpose(0, 2, 1).reshape(NQC, 128, 2, D))
        m = {"zq": zq, "zx": zx, "qtok": qtok, "xtok": xtok}
        m.update(shared)
        in_maps.append(m)

    import os
    trace = bool(os.environ.get("KERNEL_TRACE"))
    res = run_bass_kernel_spmd(nc, in_maps, list(range(N_CORES)), trace=trace)
    _NC_CACHE["exec_time_ns"] = res.exec_time_ns
    out = np.empty((B, S, D, 2), dtype=np.float32)
    for b in range(B):
        yb = res.results[b]["y"].reshape(S, 2, D)
        out[b] = yb.transpose(0, 2, 1)
    return out


if __name__ == "__main__":
    rng = np.random.default_rng(0)
    f = np.float32
    demo = {
        "x": rng.standard_normal((B, S, D, 2), dtype=f),
        "query": rng.standard_normal((B, S, D, 2), dtype=f),
        "wq": rng.standard_normal((D, D, 2), dtype=f) * 0.02,
        "bq": rng.standard_normal((D, 2), dtype=f) * 0.02,
        "wk": rng.standard_normal((D, D, 2), dtype=f) * 0.02,
        "bk": rng.standard_normal((D, 2), dtype=f) * 0.02,
        "wv": rng.standard_normal((D, D, 2), dtype=f) * 0.02,
        "bv": rng.standard_normal((D, 2), dtype=f) * 0.02,
        "w_fc": rng.standard_normal((HID, D, 2), dtype=f) * 0.02,
        "b_fc": rng.standard_normal((HID, 2), dtype=f) * 0.02,
        "w_proj": rng.standard_normal((D, HID, 2), dtype=f) * 0.02,
        "b_proj": rng.standard_normal((D, 2), dtype=f) * 0.02,
        "ln_g": np.ones((3, 2, D), dtype=f),
        "ln_b": np.zeros((3, 2, D), dtype=f),
    }
    out = kernel(**demo)
    print("out shape", out.shape)
